# revision 16
# baseline (speedup 1.0000x reference)
"""GroupedQueryAttention Trainium2 kernel (v3).

Problem shapes (hardcoded): x [2, 2048, 1024], H=16 heads, G=4 kv-groups,
head_dim=64.  out = softmax((xWq)(xWk)^T / 8) (xWv) Wo + biases.

Sharding: 8 cores, core d = (b, j) with b = d // 4, j = d % 4.
Each core computes the full attention output for batch b, query rows
[512j, 512j+512), all 16 heads — output rows are complete per core, so the
host-side gather is a pure concat.  K/V are computed per-core for the whole
batch (cheap 4x duplication).  The token axis of x^T is rolled per-core so
queries are always columns 0:512.

All matmul inputs are bf16 (PE 1 cycle/row); PSUM accumulates fp32.
Issue order is arranged so the exp stream starts ~10us in and the tail is
short:
  Q(dt0) -> per-nf-quarter: K proj then head-0 score groups (exp starts
  while x/W DMAs still stream) -> Q(dt1), head-1 scores -> V pairs
  interleaved with head-2 scores -> PV h0 -> steady state: scores(h+2)
  interleaved with PV(h), per-head denominator reciprocal, per-pair
  normalize (cast + e-vector broadcast matmuls + multiply + folded V-bias)
  -> out-projection.
V's bias is folded out of the matmul stream: softmax rows sum to 1, so
(P(V + 1 bv^T))/d = PV/d + bv — added per-partition to normalized oT.
"""

import ml_dtypes
import numpy as np

import concourse.bacc as bacc
import concourse.mybir as mybir
import concourse.tile as tile
from concourse.bass_utils import run_bass_kernel_spmd

# ---- problem constants (hardcoded per contract) ----
B, N, C = 2, 2048, 1024
H, G, HD = 16, 4, 64
DG = G * HD            # 256
NCORES = 8
SPLIT = NCORES // B    # 4 query splits per batch
NQ = N // SPLIT        # 512 query rows per core
P = 128
CT = C // P            # 8 c-chunks
KC = N // P            # 16 k-chunks
SB = 2                 # score k-chunks per PSUM batch (exp granularity)
SCALE = HD ** -0.5

F32 = mybir.dt.float32
BF16 = mybir.dt.bfloat16
NPBF = ml_dtypes.bfloat16
EXP = mybir.ActivationFunctionType.Exp

_CACHE = {}


def _build():
    nc = bacc.Bacc(None, target_bir_lowering=False)

    xbT = nc.declare_dram_parameter("xbT", [C, N], BF16, isOutput=False)
    Wq = nc.declare_dram_parameter("Wq", [C, C], BF16, isOutput=False)
    Wk = nc.declare_dram_parameter("Wk", [C, DG], BF16, isOutput=False)
    Wv = nc.declare_dram_parameter("Wv", [C, DG], BF16, isOutput=False)
    Wo = nc.declare_dram_parameter("Wo", [C, C], BF16, isOutput=False)
    bq = nc.declare_dram_parameter("bq", [C], F32, isOutput=False)
    bk = nc.declare_dram_parameter("bk", [DG], F32, isOutput=False)
    bv = nc.declare_dram_parameter("bv", [DG], F32, isOutput=False)
    bo = nc.declare_dram_parameter("bo", [C], BF16, isOutput=False)
    y = nc.declare_dram_parameter("y", [NQ, C], F32, isOutput=True)

    with tile.TileContext(nc) as tc:
        with tc.tile_pool(name="main", bufs=1) as main:
            xbTs = main.tile([P, CT, N], BF16)
            wq = main.tile([P, CT, C], BF16)
            wk = main.tile([P, CT, DG], BF16)
            wv = main.tile([P, CT, DG], BF16)
            wo = main.tile([P, CT, C], BF16)
            qT = main.tile([P, CT, NQ], BF16)         # Q^T  d-chunk x q
            kT = main.tile([P, 2, N], BF16)           # K^T  dg-chunk x k
            vA = main.tile([P, KC, G, HD + 1], BF16)  # V + ones col, per k-chunk
            oT = main.tile([P, CT, NQ], BF16)         # O^T (unnorm, then normed)
            bqk = main.tile([P, CT + 2], F32)         # bq (d-chunked) | bk
            bvo = main.tile([P, 2], F32)              # bv as two 128-chunks
            misc = main.tile([1, 3 * P], BF16)
            scr = main.tile([1, 8], F32)              # dummy-activation target
            bor = main.tile([1, C], BF16)
            ones1 = misc[0:1, 0:P]
            e_lo = misc[0:1, P:P + P]
            e_hi = misc[0:1, 2 * P:3 * P]
            bqp = bqk[:, 0:CT]
            bkp = bqk[:, CT:CT + 2]

            # ---- DMAs, in consumption order ----
            cdat = np.zeros((1, 3 * P), NPBF)
            cdat[0, 0:P] = 1.0                   # ones1
            cdat[0, P:P + HD] = 1.0              # e_lo: even heads -> rows 0..63
            cdat[0, 2 * P + HD:3 * P] = 1.0      # e_hi: odd heads -> rows 64..127
            nc.sync.dma_start(out=misc[:],
                              in_=nc.inline_tensor(cdat.view(np.uint16),
                                                   "consts")[:].bitcast(BF16))
            nc.sync.dma_start(out=bqk[:, 0:CT], in_=bq.rearrange("(t p) -> p t", p=P))
            nc.sync.dma_start(out=bqk[:, CT:CT + 2],
                              in_=bk.rearrange("(t p) -> p t", p=P))
            nc.sync.dma_start(out=bvo[:], in_=bv.rearrange("(a p) -> p a", p=P))
            nc.sync.dma_start(out=bor[:], in_=bo.rearrange("(o d) -> o d", o=1))
            for t in range(CT):
                nc.sync.dma_start(out=wq[:, t, :], in_=Wq[t * P:(t + 1) * P, :])
            for t in range(CT):
                nc.sync.dma_start(out=xbTs[:, t, 0:NQ], in_=xbT[t * P:(t + 1) * P, 0:NQ])
            for t in range(CT):
                nc.sync.dma_start(out=wk[:, t, :], in_=Wk[t * P:(t + 1) * P, :])
            for nf in range(1, 4):
                for t in range(CT):
                    nc.sync.dma_start(out=xbTs[:, t, nf * 512:(nf + 1) * 512],
                                      in_=xbT[t * P:(t + 1) * P, nf * 512:(nf + 1) * 512])
            for t in range(CT):
                nc.sync.dma_start(out=wv[:, t, :], in_=Wv[t * P:(t + 1) * P, :])
            vcol_np = np.ones((P, KC * G), NPBF)
            nc.sync.dma_start(
                out=vA[:, :, :, HD:HD + 1],
                in_=nc.inline_tensor(vcol_np.view(np.uint16),
                                     "vcol")[:].bitcast(BF16)
                .rearrange("p (k g o) -> p k g o", g=G, o=1))
            for t in range(CT):
                nc.sync.dma_start(out=wo[:, t, :], in_=Wo[t * P:(t + 1) * P, :])

            # preload the exp table set while projections run
            nc.scalar.activation(scr[0:1, 0:8], misc[0:1, 0:8], EXP)

            with tc.tile_pool(name="pp", bufs=2, space="PSUM") as pp, \
                 tc.tile_pool(name="ps", bufs=2, space="PSUM") as psp, \
                 tc.tile_pool(name="po", bufs=2, space="PSUM") as pop, \
                 tc.tile_pool(name="pt", bufs=3) as ptp, \
                 tc.tile_pool(name="rdp", bufs=2) as rdp, \
                 tc.tile_pool(name="rdb", bufs=2) as rdbp, \
                 tc.tile_pool(name="ysb", bufs=2) as ysb:

                def q_proj(dt_):
                    pq = pp.tile([P, NQ], F32, tag="pk", name=f"pq{dt_}")
                    for t in range(CT):
                        nc.tensor.matmul(
                            pq[:], wq[:, t, dt_ * P:(dt_ + 1) * P],
                            xbTs[:, t, 0:NQ], start=(t == 0), stop=(t == CT - 1))
                    nc.vector.tensor_scalar_add(qT[:, dt_, :], pq[:],
                                                bqp[:, dt_:dt_ + 1])

                def k_proj(gt, nf):
                    pk = pp.tile([P, 512], F32, tag="pk", name=f"pk{gt}_{nf}")
                    for t in range(CT):
                        nc.tensor.matmul(
                            pk[:], wk[:, t, gt * P:(gt + 1) * P],
                            xbTs[:, t, nf * 512:(nf + 1) * 512],
                            start=(t == 0), stop=(t == CT - 1))
                    nc.vector.tensor_scalar_add(
                        kT[:, gt, nf * 512:(nf + 1) * 512], pk[:], bkp[:, gt:gt + 1])

                def v_pair(j):
                    pv = pp.tile([P, 512], F32, tag="pk", name=f"pv{j}")
                    for i in range(2):
                        kc = 2 * j + i
                        for t in range(CT):
                            nc.tensor.matmul(
                                pv[:, i * DG:(i + 1) * DG],
                                xbTs[:, t, kc * P:(kc + 1) * P],
                                wv[:, t, :], start=(t == 0), stop=(t == CT - 1))
                    nc.vector.tensor_copy(
                        vA[:, 2 * j:2 * j + 2, :, 0:HD],
                        pv[:].rearrange("p (k g d) -> p k g d", k=2, g=G))

                def s_group(h, kb):
                    g = h % G
                    gt, gr = g // 2, (g % 2) * HD
                    qrow = (h % 2) * HD
                    q_h = qT[qrow:qrow + HD, h // 2, :]
                    ps = psp.tile([P, SB, NQ], F32, tag="ps", name=f"ps{h}_{kb}")
                    for i in range(SB):
                        kc = kb * SB + i
                        nc.tensor.matmul(
                            ps[:, i, :],
                            kT[gr:gr + HD, gt, kc * P:(kc + 1) * P],
                            q_h, start=True, stop=True)
                    pT = ptp.tile([P, SB, NQ], BF16, tag="pT", name=f"pT{h}_{kb}")
                    nc.scalar.activation(pT[:], ps[:], EXP, scale=SCALE)
                    return pT

                pT_of = {}

                def pv_mm(h, po, kb):
                    g = h % G
                    pT = pT_of[(h, kb)]
                    for i in range(SB):
                        kc = kb * SB + i
                        nc.tensor.matmul(
                            po[:], vA[:, kc, g, :], pT[:, i, :],
                            start=(kb == 0 and i == 0),
                            stop=(kb == KC // SB - 1 and i == SB - 1))

                rd_of = {}

                def head_out(h, po):
                    qrow = (h % 2) * HD
                    t = h // 2
                    if h % 2 == 0:
                        rd_of[t] = rdp.tile([1, 2, NQ], F32, tag="rd", name=f"rd{t}")
                    rDt = rd_of[t]
                    nc.vector.tensor_copy(oT[qrow:qrow + HD, t, :], po[0:HD, :])
                    nc.vector.tensor_copy(rDt[0:1, h % 2, :], po[HD:HD + 1, :])
                    nc.vector.reciprocal(rDt[0:1, h % 2, :], rDt[0:1, h % 2, :])

                def norm_pair(t):
                    rDt = rd_of[t]
                    rDbt = rdbp.tile([1, 2, NQ], BF16, tag="rdb", name=f"rdb{t}")
                    with nc.allow_low_precision(reason="softmax recip bf16"):
                        nc.vector.tensor_copy(
                            rDbt[:].rearrange("o a q -> o (a q)"),
                            rDt[:].rearrange("o a q -> o (a q)"))
                    pb = pp.tile([P, NQ], F32, tag="pk", name=f"pb{t}")
                    nc.tensor.matmul(pb[:], e_lo, rDbt[0:1, 0, :],
                                     start=True, stop=False)
                    nc.tensor.matmul(pb[:], e_hi, rDbt[0:1, 1, :],
                                     start=False, stop=True)
                    nc.vector.tensor_mul(oT[:, t, :], oT[:, t, :], pb[:])
                    nc.vector.tensor_scalar_add(oT[:, t, :], oT[:, t, :],
                                                bvo[:, t % 2:t % 2 + 1])

                # ---- projections ----
                q_proj(0)
                for nf in range(4):
                    k_proj(0, nf)
                    k_proj(1, nf)
                for dt_ in range(1, CT):
                    q_proj(dt_)
                for j in range(KC // 2):
                    v_pair(j)

                # ---- attention, per head; normalize per pair inline ----
                po_of = {}
                for h in range(H):
                    po_of[h] = pop.tile([HD + 1, NQ], F32, tag="po", name=f"po{h}")
                    for kb in range(KC // SB):
                        pT_of[(h, kb)] = s_group(h, kb)
                        pv_mm(h, po_of[h], kb)
                    head_out(h, po_of[h])
                    if h % 2 == 1:
                        norm_pair(h // 2)

                # ---- out-projection ----
                for m in range(NQ // P):
                    for fh in range(C // 512):
                        py = pp.tile([P, 512], F32, tag="pk", name=f"py{m}_{fh}")
                        for t in range(CT):
                            nc.tensor.matmul(
                                py[:], oT[:, t, m * P:(m + 1) * P],
                                wo[:, t, fh * 512:(fh + 1) * 512],
                                start=(t == 0), stop=False)
                        nc.tensor.matmul(py[:], ones1[:],
                                         bor[0:1, fh * 512:(fh + 1) * 512],
                                         start=False, stop=True)
                        yt = ysb.tile([P, 512], F32, tag="yt", name=f"yt{m}_{fh}")
                        nc.vector.tensor_copy(yt[:], py[:])
                        nc.sync.dma_start(
                            out=y[m * P:(m + 1) * P, fh * 512:(fh + 1) * 512],
                            in_=yt[:])

    nc.compile()
    return nc


def _get_nc():
    if "nc" not in _CACHE:
        _CACHE["nc"] = _build()
    return _CACHE["nc"]


LAST_RESULTS = None


def kernel(x, Wq, bq, Wk, bk, Wv, bv, Wo, bo, trace=False, **trace_kwargs):
    x = np.asarray(x, dtype=np.float32)
    WqB = np.ascontiguousarray(np.asarray(Wq, dtype=np.float32).astype(NPBF))
    WkB = np.ascontiguousarray(np.asarray(Wk, dtype=np.float32).astype(NPBF))
    WvB = np.ascontiguousarray(np.asarray(Wv, dtype=np.float32).astype(NPBF))
    WoB = np.ascontiguousarray(np.asarray(Wo, dtype=np.float32).astype(NPBF))
    bqF = np.ascontiguousarray(np.asarray(bq, dtype=np.float32))
    bkF = np.ascontiguousarray(np.asarray(bk, dtype=np.float32))
    bvF = np.ascontiguousarray(np.asarray(bv, dtype=np.float32))
    boB = np.ascontiguousarray(np.asarray(bo, dtype=np.float32).astype(NPBF))

    nc = _get_nc()
    in_maps = []
    for d in range(NCORES):
        b, j = d // SPLIT, d % SPLIT
        # Roll the key/token axis so this core's queries are columns 0:NQ.
        # Attention is permutation-invariant over keys, so K/V built from the
        # rolled order give identical outputs.
        xbTr = np.ascontiguousarray(
            np.roll(x[b].T, -j * NQ, axis=1).astype(NPBF))
        in_maps.append({
            "xbT": xbTr,
            "Wq": WqB, "Wk": WkB, "Wv": WvB, "Wo": WoB,
            "bq": bqF, "bk": bkF, "bv": bvF, "bo": boB,
        })

    res = run_bass_kernel_spmd(nc, in_maps, core_ids=list(range(NCORES)),
                               trace=trace, **trace_kwargs)
    global LAST_RESULTS
    LAST_RESULTS = res

    out = np.empty((B, N, C), dtype=np.float32)
    for d in range(NCORES):
        b, j = d // SPLIT, d % SPLIT
        out[b, j * NQ:(j + 1) * NQ, :] = res.results[d]["y"]
    return out


# revision 18
# speedup vs baseline: 1.0095x; 1.0095x over previous
"""GroupedQueryAttention Trainium2 kernel (v3).

Problem shapes (hardcoded): x [2, 2048, 1024], H=16 heads, G=4 kv-groups,
head_dim=64.  out = softmax((xWq)(xWk)^T / 8) (xWv) Wo + biases.

Sharding: 8 cores, core d = (b, j) with b = d // 4, j = d % 4.
Each core computes the full attention output for batch b, query rows
[512j, 512j+512), all 16 heads — output rows are complete per core, so the
host-side gather is a pure concat.  K/V are computed per-core for the whole
batch (cheap 4x duplication).  The token axis of x^T is rolled per-core so
queries are always columns 0:512.

All matmul inputs are bf16 (PE 1 cycle/row); PSUM accumulates fp32.
Issue order is arranged so the exp stream starts ~10us in and the tail is
short:
  Q(dt0) -> per-nf-quarter: K proj then head-0 score groups (exp starts
  while x/W DMAs still stream) -> Q(dt1), head-1 scores -> V pairs
  interleaved with head-2 scores -> PV h0 -> steady state: scores(h+2)
  interleaved with PV(h), per-head denominator reciprocal, per-pair
  normalize (cast + e-vector broadcast matmuls + multiply + folded V-bias)
  -> out-projection.
V's bias is folded out of the matmul stream: softmax rows sum to 1, so
(P(V + 1 bv^T))/d = PV/d + bv — added per-partition to normalized oT.
"""

import ml_dtypes
import numpy as np

import concourse.bacc as bacc
import concourse.mybir as mybir
import concourse.tile as tile
from concourse.bass_utils import run_bass_kernel_spmd

# ---- problem constants (hardcoded per contract) ----
B, N, C = 2, 2048, 1024
H, G, HD = 16, 4, 64
DG = G * HD            # 256
NCORES = 8
SPLIT = NCORES // B    # 4 query splits per batch
NQ = N // SPLIT        # 512 query rows per core
P = 128
CT = C // P            # 8 c-chunks
KC = N // P            # 16 k-chunks
SB = 2                 # score k-chunks per PSUM batch (exp granularity)
SCALE = HD ** -0.5

F32 = mybir.dt.float32
BF16 = mybir.dt.bfloat16
NPBF = ml_dtypes.bfloat16
EXP = mybir.ActivationFunctionType.Exp

_CACHE = {}


def _build():
    nc = bacc.Bacc(None, target_bir_lowering=False)

    xbT = nc.declare_dram_parameter("xbT", [C, N], BF16, isOutput=False)
    Wq = nc.declare_dram_parameter("Wq", [C, C], BF16, isOutput=False)
    Wk = nc.declare_dram_parameter("Wk", [C, DG], BF16, isOutput=False)
    Wv = nc.declare_dram_parameter("Wv", [C, DG], BF16, isOutput=False)
    Wo = nc.declare_dram_parameter("Wo", [C, C], BF16, isOutput=False)
    bq = nc.declare_dram_parameter("bq", [C], F32, isOutput=False)
    bk = nc.declare_dram_parameter("bk", [DG], F32, isOutput=False)
    bv = nc.declare_dram_parameter("bv", [DG], F32, isOutput=False)
    bo = nc.declare_dram_parameter("bo", [C], BF16, isOutput=False)
    y = nc.declare_dram_parameter("y", [NQ, C], F32, isOutput=True)

    with tile.TileContext(nc) as tc:
        with tc.tile_pool(name="main", bufs=1) as main:
            xbTs = main.tile([P, CT, N], BF16)
            wq = main.tile([P, CT, C], BF16)
            wk = main.tile([P, CT, DG], BF16)
            wv = main.tile([P, CT, DG], BF16)
            wo = main.tile([P, CT, C], BF16)
            qT = main.tile([P, CT, NQ], BF16)         # Q^T  d-chunk x q
            kT = main.tile([P, 2, N], BF16)           # K^T  dg-chunk x k
            vA = main.tile([P, KC, G, HD + 1], BF16)  # V + ones col, per k-chunk
            oT = main.tile([P, CT, NQ], BF16)         # O^T (unnorm, then normed)
            bqk = main.tile([P, CT + 2], F32)         # bq (d-chunked) | bk
            bvo = main.tile([P, 2], F32)              # bv as two 128-chunks
            misc = main.tile([1, 3 * P], BF16)
            scr = main.tile([1, 8], F32)              # dummy-activation target
            bor = main.tile([1, C], BF16)
            ones1 = misc[0:1, 0:P]
            e_lo = misc[0:1, P:P + P]
            e_hi = misc[0:1, 2 * P:3 * P]
            bqp = bqk[:, 0:CT]
            bkp = bqk[:, CT:CT + 2]

            # ---- DMAs, in consumption order ----
            cdat = np.zeros((1, 3 * P), NPBF)
            cdat[0, 0:P] = 1.0                   # ones1
            cdat[0, P:P + HD] = 1.0              # e_lo: even heads -> rows 0..63
            cdat[0, 2 * P + HD:3 * P] = 1.0      # e_hi: odd heads -> rows 64..127
            nc.sync.dma_start(out=misc[:],
                              in_=nc.inline_tensor(cdat.view(np.uint16),
                                                   "consts")[:].bitcast(BF16))
            nc.sync.dma_start(out=bqk[:, 0:CT], in_=bq.rearrange("(t p) -> p t", p=P))
            nc.sync.dma_start(out=bqk[:, CT:CT + 2],
                              in_=bk.rearrange("(t p) -> p t", p=P))
            nc.sync.dma_start(out=bvo[:], in_=bv.rearrange("(a p) -> p a", p=P))
            nc.sync.dma_start(out=bor[:], in_=bo.rearrange("(o d) -> o d", o=1))
            for t in range(CT):
                nc.sync.dma_start(out=wq[:, t, :], in_=Wq[t * P:(t + 1) * P, :])
            for t in range(CT):
                nc.sync.dma_start(out=xbTs[:, t, 0:NQ], in_=xbT[t * P:(t + 1) * P, 0:NQ])
            for t in range(CT):
                nc.sync.dma_start(out=wk[:, t, :], in_=Wk[t * P:(t + 1) * P, :])
            for nf in range(1, 4):
                for t in range(CT):
                    nc.sync.dma_start(out=xbTs[:, t, nf * 512:(nf + 1) * 512],
                                      in_=xbT[t * P:(t + 1) * P, nf * 512:(nf + 1) * 512])
            for t in range(CT):
                nc.sync.dma_start(out=wv[:, t, :], in_=Wv[t * P:(t + 1) * P, :])
            vcol_np = np.ones((P, KC * G), NPBF)
            nc.sync.dma_start(
                out=vA[:, :, :, HD:HD + 1],
                in_=nc.inline_tensor(vcol_np.view(np.uint16),
                                     "vcol")[:].bitcast(BF16)
                .rearrange("p (k g o) -> p k g o", g=G, o=1))
            for t in range(CT):
                nc.sync.dma_start(out=wo[:, t, :], in_=Wo[t * P:(t + 1) * P, :])

            # preload the exp table set while projections run
            nc.scalar.activation(scr[0:1, 0:8], misc[0:1, 0:8], EXP)

            with tc.tile_pool(name="pp", bufs=2, space="PSUM") as pp, \
                 tc.tile_pool(name="ps", bufs=2, space="PSUM") as psp, \
                 tc.tile_pool(name="po", bufs=2, space="PSUM") as pop, \
                 tc.tile_pool(name="pt", bufs=3) as ptp, \
                 tc.tile_pool(name="rdp", bufs=3) as rdp, \
                 tc.tile_pool(name="rdb", bufs=3) as rdbp, \
                 tc.tile_pool(name="ysb", bufs=2) as ysb:

                def q_proj(dt_):
                    pq = pp.tile([P, NQ], F32, tag="pk", name=f"pq{dt_}")
                    for t in range(CT):
                        nc.tensor.matmul(
                            pq[:], wq[:, t, dt_ * P:(dt_ + 1) * P],
                            xbTs[:, t, 0:NQ], start=(t == 0), stop=(t == CT - 1))
                    nc.vector.tensor_scalar_add(qT[:, dt_, :], pq[:],
                                                bqp[:, dt_:dt_ + 1])

                def k_proj(gt, nf):
                    pk = pp.tile([P, 512], F32, tag="pk", name=f"pk{gt}_{nf}")
                    for t in range(CT):
                        nc.tensor.matmul(
                            pk[:], wk[:, t, gt * P:(gt + 1) * P],
                            xbTs[:, t, nf * 512:(nf + 1) * 512],
                            start=(t == 0), stop=(t == CT - 1))
                    nc.vector.tensor_scalar_add(
                        kT[:, gt, nf * 512:(nf + 1) * 512], pk[:], bkp[:, gt:gt + 1])

                def v_pair(j):
                    pv = pp.tile([P, 512], F32, tag="pk", name=f"pv{j}")
                    for i in range(2):
                        kc = 2 * j + i
                        for t in range(CT):
                            nc.tensor.matmul(
                                pv[:, i * DG:(i + 1) * DG],
                                xbTs[:, t, kc * P:(kc + 1) * P],
                                wv[:, t, :], start=(t == 0), stop=(t == CT - 1))
                    nc.vector.tensor_copy(
                        vA[:, 2 * j:2 * j + 2, :, 0:HD],
                        pv[:].rearrange("p (k g d) -> p k g d", k=2, g=G))

                def s_group(h, kb):
                    g = h % G
                    gt, gr = g // 2, (g % 2) * HD
                    qrow = (h % 2) * HD
                    q_h = qT[qrow:qrow + HD, h // 2, :]
                    ps = psp.tile([P, SB, NQ], F32, tag="ps", name=f"ps{h}_{kb}")
                    for i in range(SB):
                        kc = kb * SB + i
                        nc.tensor.matmul(
                            ps[:, i, :],
                            kT[gr:gr + HD, gt, kc * P:(kc + 1) * P],
                            q_h, start=True, stop=True)
                    pT = ptp.tile([P, SB, NQ], BF16, tag="pT", name=f"pT{h}_{kb}")
                    nc.scalar.activation(pT[:], ps[:], EXP, scale=SCALE)
                    return pT

                pT_of = {}

                def pv_mm(h, po, kb):
                    g = h % G
                    pT = pT_of[(h, kb)]
                    for i in range(SB):
                        kc = kb * SB + i
                        nc.tensor.matmul(
                            po[:], vA[:, kc, g, :], pT[:, i, :],
                            start=(kb == 0 and i == 0),
                            stop=(kb == KC // SB - 1 and i == SB - 1))

                rd_of = {}

                def head_out(h, po):
                    qrow = (h % 2) * HD
                    t = h // 2
                    if h % 2 == 0:
                        rd_of[t] = rdp.tile([1, 2, NQ], F32, tag="rd", name=f"rd{t}")
                    rDt = rd_of[t]
                    nc.vector.tensor_copy(oT[qrow:qrow + HD, t, :], po[0:HD, :])
                    nc.vector.tensor_copy(rDt[0:1, h % 2, :], po[HD:HD + 1, :])
                    nc.vector.reciprocal(rDt[0:1, h % 2, :], rDt[0:1, h % 2, :])

                def norm_pair(t):
                    rDt = rd_of[t]
                    rDbt = rdbp.tile([1, 2, NQ], BF16, tag="rdb", name=f"rdb{t}")
                    with nc.allow_low_precision(reason="softmax recip bf16"):
                        nc.vector.tensor_copy(
                            rDbt[:].rearrange("o a q -> o (a q)"),
                            rDt[:].rearrange("o a q -> o (a q)"))
                    pb = pp.tile([P, NQ], F32, tag="pk", name=f"pb{t}")
                    nc.tensor.matmul(pb[:], e_lo, rDbt[0:1, 0, :],
                                     start=True, stop=False)
                    nc.tensor.matmul(pb[:], e_hi, rDbt[0:1, 1, :],
                                     start=False, stop=True)
                    nc.vector.tensor_mul(oT[:, t, :], oT[:, t, :], pb[:])
                    nc.vector.tensor_scalar_add(oT[:, t, :], oT[:, t, :],
                                                bvo[:, t % 2:t % 2 + 1])

                # ---- projections ----
                q_proj(0)
                for nf in range(4):
                    k_proj(0, nf)
                    k_proj(1, nf)
                for dt_ in range(1, CT):
                    q_proj(dt_)
                for j in range(KC // 2):
                    v_pair(j)

                # ---- attention, per head; normalize pair p after head 2p+3
                # (the 2-head delay keeps TensorE from stalling on the DVE
                # reciprocal/cast chain when it reaches the broadcast matmuls)
                po_of = {}
                for h in range(H):
                    po_of[h] = pop.tile([HD + 1, NQ], F32, tag="po", name=f"po{h}")
                    for kb in range(KC // SB):
                        pT_of[(h, kb)] = s_group(h, kb)
                        pv_mm(h, po_of[h], kb)
                    head_out(h, po_of[h])
                    if h % 2 == 1 and h >= 3:
                        norm_pair((h - 3) // 2)
                norm_pair(H // 2 - 1)

                # ---- out-projection ----
                for m in range(NQ // P):
                    for fh in range(C // 512):
                        py = pp.tile([P, 512], F32, tag="pk", name=f"py{m}_{fh}")
                        for t in range(CT):
                            nc.tensor.matmul(
                                py[:], oT[:, t, m * P:(m + 1) * P],
                                wo[:, t, fh * 512:(fh + 1) * 512],
                                start=(t == 0), stop=False)
                        nc.tensor.matmul(py[:], ones1[:],
                                         bor[0:1, fh * 512:(fh + 1) * 512],
                                         start=False, stop=True)
                        yt = ysb.tile([P, 512], F32, tag="yt", name=f"yt{m}_{fh}")
                        nc.vector.tensor_copy(yt[:], py[:])
                        nc.sync.dma_start(
                            out=y[m * P:(m + 1) * P, fh * 512:(fh + 1) * 512],
                            in_=yt[:])

    nc.compile()
    return nc


def _get_nc():
    if "nc" not in _CACHE:
        _CACHE["nc"] = _build()
    return _CACHE["nc"]


LAST_RESULTS = None


def kernel(x, Wq, bq, Wk, bk, Wv, bv, Wo, bo, trace=False, **trace_kwargs):
    x = np.asarray(x, dtype=np.float32)
    WqB = np.ascontiguousarray(np.asarray(Wq, dtype=np.float32).astype(NPBF))
    WkB = np.ascontiguousarray(np.asarray(Wk, dtype=np.float32).astype(NPBF))
    WvB = np.ascontiguousarray(np.asarray(Wv, dtype=np.float32).astype(NPBF))
    WoB = np.ascontiguousarray(np.asarray(Wo, dtype=np.float32).astype(NPBF))
    bqF = np.ascontiguousarray(np.asarray(bq, dtype=np.float32))
    bkF = np.ascontiguousarray(np.asarray(bk, dtype=np.float32))
    bvF = np.ascontiguousarray(np.asarray(bv, dtype=np.float32))
    boB = np.ascontiguousarray(np.asarray(bo, dtype=np.float32).astype(NPBF))

    nc = _get_nc()
    in_maps = []
    for d in range(NCORES):
        b, j = d // SPLIT, d % SPLIT
        # Roll the key/token axis so this core's queries are columns 0:NQ.
        # Attention is permutation-invariant over keys, so K/V built from the
        # rolled order give identical outputs.
        xbTr = np.ascontiguousarray(
            np.roll(x[b].T, -j * NQ, axis=1).astype(NPBF))
        in_maps.append({
            "xbT": xbTr,
            "Wq": WqB, "Wk": WkB, "Wv": WvB, "Wo": WoB,
            "bq": bqF, "bk": bkF, "bv": bvF, "bo": boB,
        })

    res = run_bass_kernel_spmd(nc, in_maps, core_ids=list(range(NCORES)),
                               trace=trace, **trace_kwargs)
    global LAST_RESULTS
    LAST_RESULTS = res

    out = np.empty((B, N, C), dtype=np.float32)
    for d in range(NCORES):
        b, j = d // SPLIT, d % SPLIT
        out[b, j * NQ:(j + 1) * NQ, :] = res.results[d]["y"]
    return out


# revision 19
# speedup vs baseline: 1.4004x; 1.3873x over previous
"""GroupedQueryAttention Trainium2 kernel (v4).

Problem shapes (hardcoded): x [2, 2048, 1024], H=16 heads, G=4 kv-groups,
head_dim=64.  out = softmax((xWq)(xWk)^T / 8) (xWv) Wo + biases.

Sharding: 8 cores, core d = (b, j) with b = d // 4, j = d % 4.
Each core computes the full attention output for batch b, query rows
[512j, 512j+512), all 16 heads — the host-side gather is a pure concat.
K/V are computed per-core for the whole batch (cheap 4x duplication).
The token axis of x^T is rolled per-core so queries are always columns
0:512 (attention is permutation-invariant over keys).

All matmul inputs are bf16 (PE 1 cycle/row when warm); PSUM is fp32.
The PE clock-gate (HAM) halves the PE clock after any ~3.4us idle window
and only re-warms after a fully-busy window, so the schedule keeps the
TensorE stream gap-free: DMAs are issued in consumption order (weights
for Q first, then the first query-block of x, ...), the exp table set is
preloaded via a dummy activation, the per-pair reciprocal/cast chain runs
on DVE strictly off the TensorE path during attention, and the tail
issues the already-satisfied broadcast matmuls first.
V's bias is folded out of the matmul stream: softmax rows sum to 1, so
(P(V + 1 bv^T))/d = PV/d + bv — added per-partition after normalize.
"""

import ml_dtypes
import numpy as np

import concourse.bacc as bacc
import concourse.mybir as mybir
import concourse.tile as tile
from concourse.bass_utils import run_bass_kernel_spmd

# ---- problem constants (hardcoded per contract) ----
B, N, C = 2, 2048, 1024
H, G, HD = 16, 4, 64
DG = G * HD            # 256
NCORES = 8
SPLIT = NCORES // B    # 4 query splits per batch
NQ = N // SPLIT        # 512 query rows per core
P = 128
CT = C // P            # 8 c-chunks
KC = N // P            # 16 k-chunks
SB = 2                 # score k-chunks per PSUM batch (exp granularity)
SCALE = HD ** -0.5

F32 = mybir.dt.float32
BF16 = mybir.dt.bfloat16
NPBF = ml_dtypes.bfloat16
EXP = mybir.ActivationFunctionType.Exp

_CACHE = {}


def _build():
    nc = bacc.Bacc(None, target_bir_lowering=False)

    xbT = nc.declare_dram_parameter("xbT", [C, N], BF16, isOutput=False)
    Wq = nc.declare_dram_parameter("Wq", [C, C], BF16, isOutput=False)
    Wk = nc.declare_dram_parameter("Wk", [C, DG], BF16, isOutput=False)
    Wv = nc.declare_dram_parameter("Wv", [C, DG], BF16, isOutput=False)
    Wo = nc.declare_dram_parameter("Wo", [C, C], BF16, isOutput=False)
    bq = nc.declare_dram_parameter("bq", [C], F32, isOutput=False)
    bk = nc.declare_dram_parameter("bk", [DG], F32, isOutput=False)
    bv = nc.declare_dram_parameter("bv", [DG], F32, isOutput=False)
    bo = nc.declare_dram_parameter("bo", [C], BF16, isOutput=False)
    y = nc.declare_dram_parameter("y", [NQ, C], F32, isOutput=True)

    with tile.TileContext(nc) as tc:
        with tc.tile_pool(name="main", bufs=1) as main:
            xbTs = main.tile([P, CT, N], BF16)
            wq = main.tile([P, CT, C], BF16)
            wk = main.tile([P, CT, DG], BF16)
            wv = main.tile([P, CT, DG], BF16)
            wo = main.tile([P, CT, C], BF16)
            qT = main.tile([P, CT, NQ], BF16)         # Q^T  d-chunk x q
            kT = main.tile([P, 2, N], BF16)           # K^T  dg-chunk x k
            vA = main.tile([P, KC, G, HD + 1], BF16)  # V + ones col, per k-chunk
            oT = main.tile([P, CT, NQ], BF16)         # O^T (unnorm, then normed)
            rDb = main.tile([1, H, NQ], BF16)         # per-head recips (bf16)
            bqk = main.tile([P, CT + 2], F32)         # bq (d-chunked) | bk
            bvo = main.tile([P, 2], F32)              # bv as two 128-chunks
            misc = main.tile([1, 3 * P], BF16)
            scr = main.tile([1, 8], F32)              # dummy-activation target
            bor = main.tile([1, C], BF16)
            ones1 = misc[0:1, 0:P]
            e_lo = misc[0:1, P:P + P]
            e_hi = misc[0:1, 2 * P:3 * P]
            bqp = bqk[:, 0:CT]
            bkp = bqk[:, CT:CT + 2]

            # ---- DMAs, in consumption order ----
            cdat = np.zeros((1, 3 * P), NPBF)
            cdat[0, 0:P] = 1.0                   # ones1
            cdat[0, P:P + HD] = 1.0              # e_lo: even heads -> rows 0..63
            cdat[0, 2 * P + HD:3 * P] = 1.0      # e_hi: odd heads -> rows 64..127
            nc.sync.dma_start(out=misc[:],
                              in_=nc.inline_tensor(cdat.view(np.uint16),
                                                   "consts")[:].bitcast(BF16))
            nc.sync.dma_start(out=bqk[:, 0:CT], in_=bq.rearrange("(t p) -> p t", p=P))
            nc.sync.dma_start(out=bqk[:, CT:CT + 2],
                              in_=bk.rearrange("(t p) -> p t", p=P))
            nc.sync.dma_start(out=bvo[:], in_=bv.rearrange("(a p) -> p a", p=P))
            nc.sync.dma_start(out=bor[:], in_=bo.rearrange("(o d) -> o d", o=1))
            for t in range(CT):
                nc.sync.dma_start(out=wq[:, t, :], in_=Wq[t * P:(t + 1) * P, :])
            for t in range(CT):
                nc.sync.dma_start(out=xbTs[:, t, 0:NQ],
                                  in_=xbT[t * P:(t + 1) * P, 0:NQ])
            for t in range(CT):
                nc.sync.dma_start(out=wk[:, t, :], in_=Wk[t * P:(t + 1) * P, :])
            for nf in range(1, 4):
                for t in range(CT):
                    nc.sync.dma_start(
                        out=xbTs[:, t, nf * 512:(nf + 1) * 512],
                        in_=xbT[t * P:(t + 1) * P, nf * 512:(nf + 1) * 512])
            for t in range(CT):
                nc.sync.dma_start(out=wv[:, t, :], in_=Wv[t * P:(t + 1) * P, :])
            vcol_np = np.ones((P, KC * G), NPBF)
            nc.sync.dma_start(
                out=vA[:, :, :, HD:HD + 1],
                in_=nc.inline_tensor(vcol_np.view(np.uint16),
                                     "vcol")[:].bitcast(BF16)
                .rearrange("p (k g o) -> p k g o", g=G, o=1))
            for t in range(CT):
                nc.sync.dma_start(out=wo[:, t, :], in_=Wo[t * P:(t + 1) * P, :])

            # preload the exp table set while projections run
            nc.scalar.activation(scr[0:1, 0:8], misc[0:1, 0:8], EXP)

            # ---------------- projections ----------------
            with tc.tile_pool(name="pp", bufs=2, space="PSUM") as pp:
                pq = pp.tile([P, NQ], F32, tag="pk", name="pq0")
                for t in range(CT):
                    nc.tensor.matmul(
                        pq[:], wq[:, t, 0:P],
                        xbTs[:, t, 0:NQ], start=(t == 0), stop=(t == CT - 1))
                nc.vector.tensor_scalar_add(qT[:, 0, :], pq[:], bqp[:, 0:1])

                for nf in range(4):
                    for gt in range(2):
                        pk = pp.tile([P, 512], F32, tag="pk", name=f"pk{gt}_{nf}")
                        for t in range(CT):
                            nc.tensor.matmul(
                                pk[:], wk[:, t, gt * P:(gt + 1) * P],
                                xbTs[:, t, nf * 512:(nf + 1) * 512],
                                start=(t == 0), stop=(t == CT - 1))
                        nc.vector.tensor_scalar_add(
                            kT[:, gt, nf * 512:(nf + 1) * 512], pk[:],
                            bkp[:, gt:gt + 1])

                for dt_ in range(1, CT):
                    pq2 = pp.tile([P, NQ], F32, tag="pk", name=f"pq{dt_}")
                    for t in range(CT):
                        nc.tensor.matmul(
                            pq2[:], wq[:, t, dt_ * P:(dt_ + 1) * P],
                            xbTs[:, t, 0:NQ], start=(t == 0), stop=(t == CT - 1))
                    nc.vector.tensor_scalar_add(qT[:, dt_, :], pq2[:],
                                                bqp[:, dt_:dt_ + 1])

                for j in range(KC // 2):
                    pv = pp.tile([P, 512], F32, tag="pk", name=f"pv{j}")
                    for i in range(2):
                        kc = 2 * j + i
                        for t in range(CT):
                            nc.tensor.matmul(
                                pv[:, i * DG:(i + 1) * DG],
                                xbTs[:, t, kc * P:(kc + 1) * P],
                                wv[:, t, :], start=(t == 0), stop=(t == CT - 1))
                    nc.vector.tensor_copy(
                        vA[:, 2 * j:2 * j + 2, :, 0:HD],
                        pv[:].rearrange("p (k g d) -> p k g d", k=2, g=G))

            # ---------------- attention ----------------
            with tc.tile_pool(name="pt", bufs=3) as ptp, \
                 tc.tile_pool(name="ps", bufs=3, space="PSUM") as psp, \
                 tc.tile_pool(name="po", bufs=2, space="PSUM") as pop, \
                 tc.tile_pool(name="rdp", bufs=2) as rdp:
                rd_of = {}
                for h in range(H):
                    g = h % G
                    gt, gr = g // 2, (g % 2) * HD
                    qrow = (h % 2) * HD
                    t = h // 2
                    q_h = qT[qrow:qrow + HD, t, :]                # [64, 512]
                    po = pop.tile([HD + 1, NQ], F32, tag="po", name=f"po{h}")
                    for kb in range(KC // SB):
                        ps = psp.tile([P, SB, NQ], F32, tag="ps", name=f"ps{h}_{kb}")
                        for i in range(SB):
                            kc = kb * SB + i
                            nc.tensor.matmul(
                                ps[:, i, :],
                                kT[gr:gr + HD, gt, kc * P:(kc + 1) * P],
                                q_h, start=True, stop=True)
                        pT = ptp.tile([P, SB, NQ], BF16, tag="pT", name=f"pT{h}_{kb}")
                        nc.scalar.activation(pT[:], ps[:], EXP, scale=SCALE)
                        for i in range(SB):
                            kc = kb * SB + i
                            nc.tensor.matmul(
                                po[:], vA[:, kc, g, :], pT[:, i, :],
                                start=(kb == 0 and i == 0),
                                stop=(kb == KC // SB - 1 and i == SB - 1))
                    # DVE-only epilogue: stash output, build reciprocal
                    nc.vector.tensor_copy(oT[qrow:qrow + HD, t, :], po[0:HD, :])
                    if h % 2 == 0:
                        rd_of[t] = rdp.tile([1, 2, NQ], F32, tag="rd", name=f"rd{t}")
                    rDt = rd_of[t]
                    nc.vector.tensor_copy(rDt[0:1, h % 2, :], po[HD:HD + 1, :])
                    nc.vector.reciprocal(rDt[0:1, h % 2, :], rDt[0:1, h % 2, :])
                    if h % 2 == 1:
                        with nc.allow_low_precision(reason="softmax recip bf16"):
                            nc.vector.tensor_copy(
                                rDb[0:1, 2 * t:2 * t + 2, :]
                                .rearrange("o a q -> o (a q)"),
                                rDt[:].rearrange("o a q -> o (a q)"))

            # ---------------- normalize + out-projection ----------------
            with tc.tile_pool(name="pb", bufs=2, space="PSUM") as pbp, \
                 tc.tile_pool(name="ysb", bufs=2) as ysb:
                for t in range(CT):
                    pb = pbp.tile([P, NQ], F32, tag="pb", name=f"pb{t}")
                    nc.tensor.matmul(pb[:], e_lo, rDb[0:1, 2 * t, :],
                                     start=True, stop=False)
                    nc.tensor.matmul(pb[:], e_hi, rDb[0:1, 2 * t + 1, :],
                                     start=False, stop=True)
                    nc.vector.tensor_mul(oT[:, t, :], oT[:, t, :], pb[:])
                    nc.vector.tensor_scalar_add(oT[:, t, :], oT[:, t, :],
                                                bvo[:, t % 2:t % 2 + 1])

                for m in range(NQ // P):
                    for fh in range(C // 512):
                        py = pbp.tile([P, 512], F32, tag="pb", name=f"py{m}_{fh}")
                        for t in range(CT):
                            nc.tensor.matmul(
                                py[:], oT[:, t, m * P:(m + 1) * P],
                                wo[:, t, fh * 512:(fh + 1) * 512],
                                start=(t == 0), stop=False)
                        nc.tensor.matmul(py[:], ones1[:],
                                         bor[0:1, fh * 512:(fh + 1) * 512],
                                         start=False, stop=True)
                        yt = ysb.tile([P, 512], F32, tag="yt", name=f"yt{m}_{fh}")
                        nc.vector.tensor_copy(yt[:], py[:])
                        nc.sync.dma_start(
                            out=y[m * P:(m + 1) * P, fh * 512:(fh + 1) * 512],
                            in_=yt[:])

    nc.compile()
    return nc


def _get_nc():
    if "nc" not in _CACHE:
        _CACHE["nc"] = _build()
    return _CACHE["nc"]


LAST_RESULTS = None


def kernel(x, Wq, bq, Wk, bk, Wv, bv, Wo, bo, trace=False, **trace_kwargs):
    x = np.asarray(x, dtype=np.float32)
    WqB = np.ascontiguousarray(np.asarray(Wq, dtype=np.float32).astype(NPBF))
    WkB = np.ascontiguousarray(np.asarray(Wk, dtype=np.float32).astype(NPBF))
    WvB = np.ascontiguousarray(np.asarray(Wv, dtype=np.float32).astype(NPBF))
    WoB = np.ascontiguousarray(np.asarray(Wo, dtype=np.float32).astype(NPBF))
    bqF = np.ascontiguousarray(np.asarray(bq, dtype=np.float32))
    bkF = np.ascontiguousarray(np.asarray(bk, dtype=np.float32))
    bvF = np.ascontiguousarray(np.asarray(bv, dtype=np.float32))
    boB = np.ascontiguousarray(np.asarray(bo, dtype=np.float32).astype(NPBF))

    nc = _get_nc()
    in_maps = []
    for d in range(NCORES):
        b, j = d // SPLIT, d % SPLIT
        # Roll the key/token axis so this core's queries are columns 0:NQ.
        xbTr = np.ascontiguousarray(
            np.roll(x[b].T, -j * NQ, axis=1).astype(NPBF))
        in_maps.append({
            "xbT": xbTr,
            "Wq": WqB, "Wk": WkB, "Wv": WvB, "Wo": WoB,
            "bq": bqF, "bk": bkF, "bv": bvF, "bo": boB,
        })

    res = run_bass_kernel_spmd(nc, in_maps, core_ids=list(range(NCORES)),
                               trace=trace, **trace_kwargs)
    global LAST_RESULTS
    LAST_RESULTS = res

    out = np.empty((B, N, C), dtype=np.float32)
    for d in range(NCORES):
        b, j = d // SPLIT, d % SPLIT
        out[b, j * NQ:(j + 1) * NQ, :] = res.results[d]["y"]
    return out
